# revision 10
# baseline (speedup 1.0000x reference)
"""Trainium2 Bass kernel v7 for nn_BertCLModel (contrastive + pairwise-MLP BCE).

Data-parallel over pair rows: core c owns i-values {8t+c, t=0..15} (interleaved
so row t only needs j >= 8t: per-t j-window w_t = 512-8t trims ~12% of work).

Host prep (replicated, tiny vs the ~8 GFLOP pair-MLP kept on device):
  z = normalize(emb); BT = W1b @ z.T (bf16), A = W1a @ z[:128].T,
  ab = A+b1 per-core columns (f32 bits carried inside the bthA bf16 blob);
  fp8 z slices for the S matmul; w2t bf16 (x32); w3 DR stationaries (x64,
  m=16, live col t%8) so logits land on psum partition t%8 of half-tile A/B.

Device per core:
  loop t: h1 = relu(BT[:, 8t:] + ab[t]) (DVE, two halves);
  stage2: 4 bf16 matmuls -> h2 psum; h2q = 32*relu(.+b2) fp8 (ACT + DVE/ACT
  alternating); stage3: 1 fp8 DR matmul accumulating into lgA (t<8) / lgB
  psum [8,512] (row t%8, cols 8t:512; cols <8t stay zero).
  lgA is copied+DMA'd mid-loop; only lgB rides the tail.
  S = (16z)'(16z) rows 0:128, 64-col slice per core (3 DR matmuls) -> f32 out.
  All DMAs ride the sync HWDGE ring (ACT queue stays compute-only).

Host combine: exact closs (exp/log in f64 from S) + exact BCE from logits.
"""

import numpy as np
import ml_dtypes

import concourse.bacc as bacc
import concourse.mybir as mybir
import concourse.tile as tile
from concourse.bass_utils import run_bass_kernel_spmd

F32 = mybir.dt.float32
BF16 = mybir.dt.bfloat16
F8 = mybir.dt.float8e4
AF = mybir.ActivationFunctionType
ALU = mybir.AluOpType
DR = mybir.MatmulPerfMode.DoubleRow

NPF8 = ml_dtypes.float8_e4m3fn
NPBF = ml_dtypes.bfloat16

B, D, H = 512, 768, 256
N_ROWS = 128
M_POS = 256
NCORES = 8
TPC = 16
NPAIRS = 57280
SZ = 16.0        # z fp8 scale
SW2 = 32.0       # W2 bf16 scale (h2q = 32*relu(h2pre))
SW3 = 64.0       # W3 fp8 scale
SL = SW2 * SW3   # logit raw scale = 2048

_STATE = {}


def _wt(t):
    return B - 8 * t


def _build():
    nc = bacc.Bacc("TRN2", target_bir_lowering=False, debug=False,
                   num_devices=NCORES)

    btha_d = nc.dram_tensor("btha", [128, B + 80], BF16, kind="ExternalInput")
    w2t_d = nc.dram_tensor("w2t", [128, 2 * H], BF16, kind="ExternalInput")
    bthb_d = nc.dram_tensor("bthb", [128, B], BF16, kind="ExternalInput")
    w3s_d = nc.dram_tensor("w3s", [128, 512], F8, kind="ExternalInput")
    zs_d = nc.dram_tensor("zs", [128, 1152], F8, kind="ExternalInput")
    sout_d = nc.dram_tensor("sout", [128, 64], F32, kind="ExternalOutput")
    lgout_d = nc.dram_tensor("lgout", [TPC, B], BF16, kind="ExternalOutput")

    with tile.TileContext(nc) as tc:
        with (
            tc.tile_pool(name="io", bufs=1) as io,
            tc.tile_pool(name="big", bufs=1) as big,
            tc.tile_pool(name="h1ap", bufs=3) as h1ap,
            tc.tile_pool(name="h1bp", bufs=3) as h1bp,
            tc.tile_pool(name="h2qp", bufs=3) as h2qp,
            tc.tile_pool(name="ps", bufs=1, space="PSUM") as ps,
        ):
            # ---------- input DMAs, priority order on the sync HWDGE ring ----
            btha = io.tile([128, B + 80], BF16, name="btha", tag="btha")
            nc.sync.dma_start(btha[:], btha_d[:])
            w2t = io.tile([128, 2 * H], BF16, name="w2t", tag="w2t")
            nc.sync.dma_start(w2t[:], w2t_d[:])
            bthb = io.tile([128, B], BF16, name="bthb", tag="bthb")
            nc.sync.dma_start(bthb[:], bthb_d[:])
            w3s = io.tile([128, 512], F8, name="w3s", tag="w3s")
            nc.sync.dma_start(w3s[:], w3s_d[:])
            zs = io.tile([128, 1152], F8, name="zs", tag="zs")
            nc.sync.dma_start(zs[:], zs_d[:])

            BT = [btha[:, 0:B], bthb[:]]
            bvec = btha[:, B:B + 80].bitcast(F32)       # [128, 40] f32 bits
            ab = [bvec[:, 0:TPC], bvec[:, TPC:2 * TPC]]
            b2c = [bvec[:, 32:33], bvec[:, 33:34]]

            # ---------- warm the ACT table set (relu/copy only: one set) ----
            warm = big.tile([1, 2], F32, name="warm", tag="warm")
            nc.vector.memset(warm[:], 0.0)
            nc.scalar.activation(warm[:, 0:1], warm[:, 1:2], AF.Relu)
            nc.scalar.activation(warm[:, 0:1], warm[:, 1:2], AF.Copy)

            # ---------- PE HAM warmup: fine-grained N=128 matmuls ----------
            wtile = big.tile([128, 128], BF16, name="wtile", tag="wtile")
            nc.vector.memset(wtile[:], 0.0)
            wu_ps = ps.tile([128, 128], F32, name="wu_ps", tag="pS")
            NWU = 24
            for i in range(NWU):
                nc.tensor.matmul(wu_ps[:], wtile[:], wtile[:],
                                 start=(i == 0), stop=(i == NWU - 1))

            # ---------- per-iteration emitters ----------
            out_sc = big.tile([128, 64], F32, name="out_sc", tag="out_sc")
            lgs = [big.tile([8, B], BF16, name=f"lgs{h}", tag=f"lgs{h}")
                   for h in range(2)]
            lg_ps = [ps.tile([16, B], F32, name=f"lg{h}", tag=f"lg{h}")
                     for h in range(2)]
            h1as = [None] * TPC
            h1bs = [None] * TPC
            h2qs = {}
            ctx = {}

            def emit_h1(t):
                w = _wt(t)
                h1a = h1ap.tile([128, w], BF16, name=f"h1a_{t}", tag="h1a")
                nc.vector.tensor_scalar(h1a[:], BT[0][:, 8 * t:B],
                                        ab[0][:, t:t + 1], 0.0,
                                        op0=ALU.add, op1=ALU.max)
                h1b = h1bp.tile([128, w], BF16, name=f"h1b_{t}", tag="h1b")
                nc.vector.tensor_scalar(h1b[:], BT[1][:, 8 * t:B],
                                        ab[1][:, t:t + 1], 0.0,
                                        op0=ALU.add, op1=ALU.max)
                h1as[t], h1bs[t] = h1a, h1b

            def emit_stage2(t):
                w = _wt(t)
                h2_ps = [ps.tile([128, w], F32, name=f"h2_{t}_{ho}",
                                 tag=f"h{(2 * t + ho) % 4}") for ho in range(2)]
                # hi=0 (h1a) for both ho first: gives h1b extra slack
                for hi in range(2):
                    src = (h1as[t] if hi == 0 else h1bs[t])[:]
                    for ho in range(2):
                        nc.tensor.matmul(
                            h2_ps[ho][:],
                            w2t[:, hi * H + ho * 128:hi * H + (ho + 1) * 128],
                            src,
                            start=(hi == 0), stop=(hi == 1))
                h1as[t] = h1bs[t] = None
                ctx[("h2ps", t)] = h2_ps

            def emit_h2q(t):
                w = _wt(t)
                h2_ps = ctx.pop(("h2ps", t))
                h2q = h2qp.tile([128, 2 * w], F8, name=f"h2q_{t}", tag="h2q")
                nc.scalar.activation(h2q[:, 0:w], h2_ps[0][:], AF.Relu,
                                     bias=b2c[0])
                if t % 2 == 0:
                    nc.vector.tensor_scalar(h2q[:, w:2 * w], h2_ps[1][:],
                                            b2c[1], 0.0,
                                            op0=ALU.add, op1=ALU.max)
                else:
                    nc.scalar.activation(h2q[:, w:2 * w], h2_ps[1][:], AF.Relu,
                                         bias=b2c[1])
                h2qs[t] = h2q

            def emit_stage3(t):
                tgt = lg_ps[t // 8]
                nc.tensor.matmul(
                    tgt[:, 8 * t:B],
                    w3s[:, t * 32:(t + 1) * 32]
                    .rearrange("p (i m) -> p i m", i=2),
                    h2qs[t][:].rearrange("p (i n) -> p i n", i=2),
                    start=(t % 8 == 0), stop=(t % 8 == 7), perf_mode=DR)
                h2qs[t] = None

            def emit_lg_out(h):
                nc.vector.tensor_copy(lgs[h][:, 0:256], lg_ps[h][0:8, 0:256])
                nc.scalar.copy(lgs[h][:, 256:B], lg_ps[h][0:8, 256:B])
                nc.sync.dma_start(lgout_d[8 * h:8 * (h + 1), :], lgs[h][:])

            def emit_S_mm():
                g_ps = ps.tile([128, 64], F32, name="g_ps", tag="pS")
                for k2 in range(3):
                    nc.tensor.matmul(
                        g_ps[:],
                        zs[:, k2 * 256:(k2 + 1) * 256]
                        .rearrange("p (i m) -> p i m", i=2),
                        zs[:, 768 + k2 * 128:768 + (k2 + 1) * 128]
                        .rearrange("p (i n) -> p i n", i=2),
                        start=(k2 == 0), stop=(k2 == 2), perf_mode=DR)
                ctx["g_ps"] = g_ps

            def emit_S_out():
                nc.vector.tensor_copy(out_sc[:], ctx["g_ps"][:])
                nc.sync.dma_start(sout_d[:], out_sc[:])

            # pipeline: h1[t] | stage2[t-1] | h2q[t-2] | stage3[t-3]
            for step in range(TPC + 3):
                if step < TPC:
                    emit_h1(step)
                if 1 <= step <= TPC:
                    emit_stage2(step - 1)
                if 2 <= step <= TPC + 1:
                    emit_h2q(step - 2)
                if step >= 3:
                    emit_stage3(step - 3)
                if step == 4:
                    emit_S_mm()
                elif step == 5:
                    emit_S_out()
                elif step == 12:
                    emit_lg_out(0)
            emit_lg_out(1)

    nc.compile()
    return nc


def _in_maps(emb_in, W1, b1, W2, b2, W3, b3):
    _STATE["b3"] = float(np.asarray(b3).reshape(-1)[0])
    emb = np.asarray(emb_in, np.float32)
    z = emb / np.maximum(np.linalg.norm(emb, axis=1, keepdims=True), 1e-12)
    zT = np.ascontiguousarray(z.T)                      # [768, 512]
    W1f = np.asarray(W1, np.float32)

    # BT = W1b @ z.T  [256, 512] -> two bf16 halves (replicated)
    BTm = W1f[:, D:] @ zT
    bthb = np.ascontiguousarray(BTm[128:].astype(NPBF))

    # A = W1a @ z.T[:, :128]  [256, 128] (host; tiny)
    Am = W1f[:, :D] @ zT[:, :N_ROWS]
    b1v = np.asarray(b1, np.float32).reshape(H)
    abf = Am + b1v[:, None]                             # [256, 128]

    # w2t [128, 2H] bf16: w2t[p, hi*H+m] = 32*W2[m, hi*128+p]
    W2s = (SW2 * np.asarray(W2, np.float32).T)
    w2t = np.empty((128, 2 * H), dtype=NPBF)
    for hi in range(2):
        w2t[:, hi * H:(hi + 1) * H] = W2s[hi * 128:(hi + 1) * 128].astype(NPBF)

    # w3s [128, 512] f8: per t a [128, (i=2, m=16)] DR stationary, live col t%8
    w3s = np.zeros((128, 512), dtype=NPF8)
    W3s = (SW3 * np.asarray(W3, np.float32).reshape(H))
    for t in range(TPC):
        for i in range(2):
            w3s[:, t * 32 + i * 16 + (t % 8)] = \
                W3s[i * 128:(i + 1) * 128].astype(NPF8)

    b2v = (SW2 * np.asarray(b2, np.float32)).reshape(H)
    zf8img = np.empty((128, 6 * B), dtype=NPF8)
    for q in range(6):
        zf8img[:, q * B:(q + 1) * B] = (SZ * zT[q * 128:(q + 1) * 128]).astype(NPF8)

    # zs stat part [0:768]: k2*256 + i*128 + m  = zf8img[chunk 2k2+i][:, m]
    zs_shared = np.empty((128, 1152), dtype=NPF8)
    for k2 in range(3):
        for i in range(2):
            zs_shared[:, k2 * 256 + i * 128:k2 * 256 + (i + 1) * 128] = \
                zf8img[:, (2 * k2 + i) * B:(2 * k2 + i) * B + 128]

    maps = []
    for c in range(NCORES):
        i_vals = 8 * np.arange(TPC) + c
        bvec = np.zeros((128, 40), np.float32)
        bvec[:, 0:TPC] = abf[:128][:, i_vals]
        bvec[:, TPC:2 * TPC] = abf[128:][:, i_vals]
        bvec[:, 32] = b2v[:128]
        bvec[:, 33] = b2v[128:]
        btha = np.empty((128, B + 80), dtype=NPBF)
        btha[:, 0:B] = BTm[:128].astype(NPBF)
        btha[:, B:B + 80] = np.ascontiguousarray(bvec).view(NPBF)
        zsc = zs_shared.copy()
        for k2 in range(3):
            for i in range(2):
                zsc[:, 768 + k2 * 128 + i * 64:768 + k2 * 128 + (i + 1) * 64] = \
                    zf8img[:, (2 * k2 + i) * B + 64 * c:
                           (2 * k2 + i) * B + 64 * (c + 1)]
        maps.append({"btha": btha, "w2t": w2t, "bthb": bthb,
                     "w3s": w3s, "zs": zsc})
    return maps


def _run(in_maps, **kw):
    if "nc" not in _STATE:
        _STATE["nc"] = _build()
    return run_bass_kernel_spmd(_STATE["nc"], in_maps,
                                core_ids=list(range(NCORES)), **kw)


def _combine(results, b3=None):
    b3f = _STATE["b3"] if b3 is None else float(np.asarray(b3).reshape(-1)[0])
    # ---- closs from the f32 S slices ----
    S = np.concatenate([np.asarray(r["sout"], np.float64) for r in results],
                       axis=1) / (SZ * SZ)              # [128, 512]
    E = np.exp(2.0 * S)
    denom = E.sum(axis=1) - np.exp(2.0 * np.diagonal(S)[:N_ROWS])
    ld = np.log(denom)
    coeff = (N_ROWS - 1 - np.arange(N_ROWS)).astype(np.float64)
    Sn = S[:, :N_ROWS]
    triu = np.triu(np.ones((N_ROWS, N_ROWS)), k=1)
    closs_sum = float(coeff @ ld) - 2.0 * float((Sn * triu).sum())
    closs = (-2.0 * (N_ROWS - 1) / N_ROWS) * closs_sum

    # ---- exact BCE from raw logits ----
    j = np.arange(B)
    bce_total = 0.0
    for c in range(NCORES):
        lg = np.asarray(results[c]["lgout"], np.float64)   # [16, 512]
        i_vals = 8 * np.arange(TPC) + c
        m = j[None, :] > i_vals[:, None]
        y = (j[None, :] < M_POS).astype(np.float64)
        l = lg / SL + b3f
        bce = np.maximum(l, 0.0) - l * y + np.log1p(np.exp(-np.abs(l)))
        bce_total += float(bce[m].sum())
    eloss = bce_total / NPAIRS
    return np.float32(closs + eloss)


def kernel(emb_in, W1, b1, W2, b2, W3, b3):
    res = _run(_in_maps(emb_in, W1, b1, W2, b2, W3, b3))
    return _combine(res.results)


# revision 13
# speedup vs baseline: 1.1119x; 1.1119x over previous
"""Trainium2 Bass kernel v7 for nn_BertCLModel (contrastive + pairwise-MLP BCE).

Data-parallel over pair rows: core c owns i-values {8t+c, t=0..15} (interleaved
so row t only needs j >= 8t: per-t j-window w_t = 512-8t trims ~12% of work).

Host prep (replicated, tiny vs the ~8 GFLOP pair-MLP kept on device):
  z = normalize(emb); BT = W1b @ z.T (bf16), A = W1a @ z[:128].T,
  ab = A+b1 per-core columns (f32 bits carried inside the bthA bf16 blob);
  fp8 z slices for the S matmul; w2t bf16 (x32); w3 DR stationaries (x64,
  m=16, live col t%8) so logits land on psum partition t%8 of half-tile A/B.

Device per core:
  loop t: h1 = relu(BT[:, 8t:] + ab[t]) (DVE, two halves);
  stage2: 4 bf16 matmuls -> h2 psum; h2q = 32*relu(.+b2) fp8 (ACT + DVE/ACT
  alternating); stage3: 1 fp8 DR matmul accumulating into lgA (t<8) / lgB
  psum [8,512] (row t%8, cols 8t:512; cols <8t stay zero).
  lgA is copied+DMA'd mid-loop; only lgB rides the tail.
  S = (16z)'(16z) rows 0:128, 64-col slice per core (3 DR matmuls) -> f32 out.
  All DMAs ride the sync HWDGE ring (ACT queue stays compute-only).

Host combine: exact closs (exp/log in f64 from S) + exact BCE from logits.
"""

import numpy as np
import ml_dtypes

import concourse.bacc as bacc
import concourse.mybir as mybir
import concourse.tile as tile
from concourse.bass_utils import run_bass_kernel_spmd

F32 = mybir.dt.float32
BF16 = mybir.dt.bfloat16
F8 = mybir.dt.float8e4
AF = mybir.ActivationFunctionType
ALU = mybir.AluOpType
DR = mybir.MatmulPerfMode.DoubleRow

NPF8 = ml_dtypes.float8_e4m3fn
NPBF = ml_dtypes.bfloat16

B, D, H = 512, 768, 256
N_ROWS = 128
M_POS = 256
NCORES = 8
TPC = 16
NPAIRS = 57280
SZ = 16.0        # z fp8 scale
SW2 = 32.0       # W2 bf16 scale (h2q = 32*relu(h2pre))
SW3 = 64.0       # W3 fp8 scale
SL = SW2 * SW3   # logit raw scale = 2048

_STATE = {}


def _wt(t):
    return B - 8 * t


def _build():
    nc = bacc.Bacc("TRN2", target_bir_lowering=False, debug=False,
                   num_devices=NCORES)

    btha_d = nc.dram_tensor("btha", [128, B + 80], BF16, kind="ExternalInput")
    w2t_d = nc.dram_tensor("w2t", [128, 2 * H], BF16, kind="ExternalInput")
    bthb_d = nc.dram_tensor("bthb", [128, B], BF16, kind="ExternalInput")
    w3s_d = nc.dram_tensor("w3s", [128, 512], F8, kind="ExternalInput")
    zs_d = nc.dram_tensor("zs", [128, 1152], F8, kind="ExternalInput")
    sout_d = nc.dram_tensor("sout", [128, 64], F32, kind="ExternalOutput")
    lgout_d = nc.dram_tensor("lgout", [TPC, B], BF16, kind="ExternalOutput")

    with tile.TileContext(nc) as tc:
        with (
            tc.tile_pool(name="io", bufs=1) as io,
            tc.tile_pool(name="big", bufs=1) as big,
            tc.tile_pool(name="h1ap", bufs=3) as h1ap,
            tc.tile_pool(name="h1bp", bufs=3) as h1bp,
            tc.tile_pool(name="h2qp", bufs=3) as h2qp,
            tc.tile_pool(name="ps", bufs=1, space="PSUM") as ps,
        ):
            # ---------- input DMAs, priority order on the sync HWDGE ring ----
            btha = io.tile([128, B + 80], BF16, name="btha", tag="btha")
            nc.sync.dma_start(btha[:], btha_d[:])
            w2t = io.tile([128, 2 * H], BF16, name="w2t", tag="w2t")
            nc.sync.dma_start(w2t[:], w2t_d[:])
            bthb = io.tile([128, B], BF16, name="bthb", tag="bthb")
            nc.sync.dma_start(bthb[:], bthb_d[:])
            w3s = io.tile([128, 512], F8, name="w3s", tag="w3s")
            nc.sync.dma_start(w3s[:], w3s_d[:])
            zs = io.tile([128, 1152], F8, name="zs", tag="zs")
            nc.sync.dma_start(zs[:], zs_d[:])

            BT = [btha[:, 0:B], bthb[:]]
            bvec = btha[:, B:B + 80].bitcast(F32)       # [128, 40] f32 bits
            ab = [bvec[:, 0:TPC], bvec[:, TPC:2 * TPC]]
            b2c = [bvec[:, 32:33], bvec[:, 33:34]]

            # ---------- warm the ACT table set (relu/copy only: one set) ----
            warm = big.tile([1, 2], F32, name="warm", tag="warm")
            nc.vector.memset(warm[:], 0.0)
            nc.scalar.activation(warm[:, 0:1], warm[:, 1:2], AF.Relu)
            nc.scalar.activation(warm[:, 0:1], warm[:, 1:2], AF.Copy)

            # ---------- PE HAM warmup: fine-grained N=128 matmuls ----------
            wtile = big.tile([128, 128], BF16, name="wtile", tag="wtile")
            nc.vector.memset(wtile[:], 0.0)
            wu_ps = ps.tile([128, 128], F32, name="wu_ps", tag="pS")
            NWU = 18
            for i in range(NWU):
                nc.tensor.matmul(wu_ps[:], wtile[:], wtile[:],
                                 start=(i == 0), stop=(i == NWU - 1))

            # ---------- per-iteration emitters ----------
            out_sc = big.tile([128, 64], F32, name="out_sc", tag="out_sc")
            lgs = [big.tile([8, B], BF16, name=f"lgs{h}", tag=f"lgs{h}")
                   for h in range(2)]
            lg_ps = [ps.tile([16, B], F32, name=f"lg{h}", tag=f"lg{h}")
                     for h in range(2)]
            h1as = [None] * TPC
            h1bs = [None] * TPC
            h2qs = {}
            ctx = {}

            def emit_h1(t):
                w = _wt(t)
                h1a = h1ap.tile([128, w], BF16, name=f"h1a_{t}", tag="h1a")
                nc.vector.tensor_scalar(h1a[:], BT[0][:, 8 * t:B],
                                        ab[0][:, t:t + 1], 0.0,
                                        op0=ALU.add, op1=ALU.max)
                h1b = h1bp.tile([128, w], BF16, name=f"h1b_{t}", tag="h1b")
                nc.vector.tensor_scalar(h1b[:], BT[1][:, 8 * t:B],
                                        ab[1][:, t:t + 1], 0.0,
                                        op0=ALU.add, op1=ALU.max)
                h1as[t], h1bs[t] = h1a, h1b

            def emit_stage2(t):
                w = _wt(t)
                h2_ps = [ps.tile([128, w], F32, name=f"h2_{t}_{ho}",
                                 tag=f"h{(2 * t + ho) % 4}") for ho in range(2)]
                # hi=0 (h1a) for both ho first: gives h1b extra slack
                for hi in range(2):
                    src = (h1as[t] if hi == 0 else h1bs[t])[:]
                    for ho in range(2):
                        nc.tensor.matmul(
                            h2_ps[ho][:],
                            w2t[:, hi * H + ho * 128:hi * H + (ho + 1) * 128],
                            src,
                            start=(hi == 0), stop=(hi == 1))
                h1as[t] = h1bs[t] = None
                ctx[("h2ps", t)] = h2_ps

            def emit_h2q(t):
                w = _wt(t)
                h2_ps = ctx.pop(("h2ps", t))
                h2q = h2qp.tile([128, 2 * w], F8, name=f"h2q_{t}", tag="h2q")
                nc.scalar.activation(h2q[:, 0:w], h2_ps[0][:], AF.Relu,
                                     bias=b2c[0])
                if t % 2 == 0 or t == TPC - 1:
                    nc.vector.tensor_scalar(h2q[:, w:2 * w], h2_ps[1][:],
                                            b2c[1], 0.0,
                                            op0=ALU.add, op1=ALU.max)
                else:
                    nc.scalar.activation(h2q[:, w:2 * w], h2_ps[1][:], AF.Relu,
                                         bias=b2c[1])
                h2qs[t] = h2q

            def emit_stage3(t):
                tgt = lg_ps[t // 8]
                nc.tensor.matmul(
                    tgt[:, 8 * t:B],
                    w3s[:, t * 32:(t + 1) * 32]
                    .rearrange("p (i m) -> p i m", i=2),
                    h2qs[t][:].rearrange("p (i n) -> p i n", i=2),
                    start=(t % 8 == 0), stop=(t % 8 == 7), perf_mode=DR)
                h2qs[t] = None

            def emit_lg_out(h):
                if h == 0:
                    nc.vector.tensor_copy(lgs[h][:], lg_ps[h][0:8, :])
                else:
                    nc.scalar.copy(lgs[h][:], lg_ps[h][0:8, :])
                nc.sync.dma_start(lgout_d[8 * h:8 * (h + 1), :], lgs[h][:])

            def emit_S_mm():
                g_ps = ps.tile([128, 64], F32, name="g_ps", tag="pS")
                for k2 in range(3):
                    nc.tensor.matmul(
                        g_ps[:],
                        zs[:, k2 * 256:(k2 + 1) * 256]
                        .rearrange("p (i m) -> p i m", i=2),
                        zs[:, 768 + k2 * 128:768 + (k2 + 1) * 128]
                        .rearrange("p (i n) -> p i n", i=2),
                        start=(k2 == 0), stop=(k2 == 2), perf_mode=DR)
                ctx["g_ps"] = g_ps

            def emit_S_out():
                nc.vector.tensor_copy(out_sc[:], ctx["g_ps"][:])
                nc.sync.dma_start(sout_d[:], out_sc[:])

            # pipeline: h1[t] | stage2[t-1] | h2q[t-2] | stage3[t-3]
            for step in range(TPC + 3):
                if step < TPC:
                    emit_h1(step)
                if 1 <= step <= TPC:
                    emit_stage2(step - 1)
                if 2 <= step <= TPC + 1:
                    emit_h2q(step - 2)
                if step >= 3:
                    emit_stage3(step - 3)
                if step == 4:
                    emit_S_mm()
                elif step == 5:
                    emit_S_out()
                elif step == 12:
                    emit_lg_out(0)
            emit_lg_out(1)

    nc.compile()
    return nc


def _in_maps(emb_in, W1, b1, W2, b2, W3, b3):
    _STATE["b3"] = float(np.asarray(b3).reshape(-1)[0])
    emb = np.asarray(emb_in, np.float32)
    z = emb / np.maximum(np.linalg.norm(emb, axis=1, keepdims=True), 1e-12)
    zT = np.ascontiguousarray(z.T)                      # [768, 512]
    W1f = np.asarray(W1, np.float32)

    # BT = W1b @ z.T  [256, 512] -> two bf16 halves (replicated)
    BTm = W1f[:, D:] @ zT
    bthb = np.ascontiguousarray(BTm[128:].astype(NPBF))

    # A = W1a @ z.T[:, :128]  [256, 128] (host; tiny)
    Am = W1f[:, :D] @ zT[:, :N_ROWS]
    b1v = np.asarray(b1, np.float32).reshape(H)
    abf = Am + b1v[:, None]                             # [256, 128]

    # w2t [128, 2H] bf16: w2t[p, hi*H+m] = 32*W2[m, hi*128+p]
    W2s = (SW2 * np.asarray(W2, np.float32).T)
    w2t = np.empty((128, 2 * H), dtype=NPBF)
    for hi in range(2):
        w2t[:, hi * H:(hi + 1) * H] = W2s[hi * 128:(hi + 1) * 128].astype(NPBF)

    # w3s [128, 512] f8: per t a [128, (i=2, m=16)] DR stationary, live col t%8
    w3s = np.zeros((128, 512), dtype=NPF8)
    W3s = (SW3 * np.asarray(W3, np.float32).reshape(H))
    for t in range(TPC):
        for i in range(2):
            w3s[:, t * 32 + i * 16 + (t % 8)] = \
                W3s[i * 128:(i + 1) * 128].astype(NPF8)

    b2v = (SW2 * np.asarray(b2, np.float32)).reshape(H)
    zf8img = np.empty((128, 6 * B), dtype=NPF8)
    for q in range(6):
        zf8img[:, q * B:(q + 1) * B] = (SZ * zT[q * 128:(q + 1) * 128]).astype(NPF8)

    # zs stat part [0:768]: k2*256 + i*128 + m  = zf8img[chunk 2k2+i][:, m]
    zs_shared = np.empty((128, 1152), dtype=NPF8)
    for k2 in range(3):
        for i in range(2):
            zs_shared[:, k2 * 256 + i * 128:k2 * 256 + (i + 1) * 128] = \
                zf8img[:, (2 * k2 + i) * B:(2 * k2 + i) * B + 128]

    maps = []
    for c in range(NCORES):
        i_vals = 8 * np.arange(TPC) + c
        bvec = np.zeros((128, 40), np.float32)
        bvec[:, 0:TPC] = abf[:128][:, i_vals]
        bvec[:, TPC:2 * TPC] = abf[128:][:, i_vals]
        bvec[:, 32] = b2v[:128]
        bvec[:, 33] = b2v[128:]
        btha = np.empty((128, B + 80), dtype=NPBF)
        btha[:, 0:B] = BTm[:128].astype(NPBF)
        btha[:, B:B + 80] = np.ascontiguousarray(bvec).view(NPBF)
        zsc = zs_shared.copy()
        for k2 in range(3):
            for i in range(2):
                zsc[:, 768 + k2 * 128 + i * 64:768 + k2 * 128 + (i + 1) * 64] = \
                    zf8img[:, (2 * k2 + i) * B + 64 * c:
                           (2 * k2 + i) * B + 64 * (c + 1)]
        maps.append({"btha": btha, "w2t": w2t, "bthb": bthb,
                     "w3s": w3s, "zs": zsc})
    return maps


def _run(in_maps, **kw):
    if "nc" not in _STATE:
        _STATE["nc"] = _build()
    return run_bass_kernel_spmd(_STATE["nc"], in_maps,
                                core_ids=list(range(NCORES)), **kw)


def _combine(results, b3=None):
    b3f = _STATE["b3"] if b3 is None else float(np.asarray(b3).reshape(-1)[0])
    # ---- closs from the f32 S slices ----
    S = np.concatenate([np.asarray(r["sout"], np.float64) for r in results],
                       axis=1) / (SZ * SZ)              # [128, 512]
    E = np.exp(2.0 * S)
    denom = E.sum(axis=1) - np.exp(2.0 * np.diagonal(S)[:N_ROWS])
    ld = np.log(denom)
    coeff = (N_ROWS - 1 - np.arange(N_ROWS)).astype(np.float64)
    Sn = S[:, :N_ROWS]
    triu = np.triu(np.ones((N_ROWS, N_ROWS)), k=1)
    closs_sum = float(coeff @ ld) - 2.0 * float((Sn * triu).sum())
    closs = (-2.0 * (N_ROWS - 1) / N_ROWS) * closs_sum

    # ---- exact BCE from raw logits ----
    j = np.arange(B)
    bce_total = 0.0
    for c in range(NCORES):
        lg = np.asarray(results[c]["lgout"], np.float64)   # [16, 512]
        i_vals = 8 * np.arange(TPC) + c
        m = j[None, :] > i_vals[:, None]
        y = (j[None, :] < M_POS).astype(np.float64)
        l = lg / SL + b3f
        bce = np.maximum(l, 0.0) - l * y + np.log1p(np.exp(-np.abs(l)))
        bce_total += float(bce[m].sum())
    eloss = bce_total / NPAIRS
    return np.float32(closs + eloss)


def kernel(emb_in, W1, b1, W2, b2, W3, b3):
    res = _run(_in_maps(emb_in, W1, b1, W2, b2, W3, b3))
    return _combine(res.results)
